# revision 41
# baseline (speedup 1.0000x reference)
"""Trainium2 Bass kernel for NEAT-style genome evaluation (gnn_message_passing).

Shapes are hardcoded for the problem:
  inputs [16384, 256] f32, in_idx/edge_w/edge_mask [768, 8], bias/response [768],
  out_idx [768] (scan order), output_idx [64]; output [16384, 64] f32.

Strategy: pure data-parallel over batch (2048 rows/core on 8 cores). The host
pre-transposes the input to node-major fp16 so DMA lands directly in the SBUF
value store v[chunk] = [128 nodes, 2048 batch] (no on-device marshaling).
Nodes are packed by topological level into 128-row chunks; per (level, pass)
the pre-activations accumulate in PSUM via fp16 matmuls whose [128,128]
stationary blocks embed the sparse DAG edges, and tanh(bias + response*s)
runs on the scalar engine over [128 rows, 1024 batch].  The batch is processed
in two sequential passes of 1024 columns so each chunk's PSUM tile is only
2 banks, allowing 4 live tiles and a 3-chunk cross-chunk matmul prefetch
horizon; cross-chunk ("parta") matmul quarters are greedily interleaved into
the level cascade as fillers so the tensor engine stays busy under the serial
activation chain.  Output node rows are DMA'd fp16 node-major; the host
gathers/transposes/converts.  fp16 keeps relative error ~3e-3 (tolerance 2e-2).
"""

import sys

import numpy as np

if "/opt/trn_rl_repo" not in sys.path:
    sys.path.insert(0, "/opt/trn_rl_repo")

import concourse.bacc as bacc
import concourse.mybir as mybir
from concourse.bass_utils import run_bass_kernel_spmd
from concourse.tile import TileContext

F16 = mybir.dt.float16
F32 = mybir.dt.float32

B = 16384
NUM_IN = 256
N = 1024
K = 8
NN = N - NUM_IN
NUM_OUT = 64
NCORES = 8
BC = B // NCORES          # batch rows per core (2048)
HALF = BC // 2            # batch columns per pass (1024)
NQ = 2                    # 512-wide matmul quarters per pass
HORIZON = 3               # psum prefetch horizon in chunks


def _plan(in_idx, edge_mask, edge_w, bias, response, out_idx, output_idx):
    """All host-side graph analysis; returns the constant tensors + schedule."""
    in_idx = np.asarray(in_idx)
    edge_mask = np.asarray(edge_mask).astype(bool)
    edge_w = np.asarray(edge_w).astype(np.float32)
    bias = np.asarray(bias).astype(np.float32)
    response = np.asarray(response).astype(np.float32)
    out_idx = np.asarray(out_idx)
    output_idx = np.asarray(output_idx)

    # scan write position of each node (reference writes out_idx[r] at step r)
    write_pos = np.full(N, -1, dtype=np.int64)
    for r in range(NN):
        write_pos[out_idx[r]] = r

    # valid edges: mask set AND source reads a value written before this step
    valid = np.zeros((NN, K), dtype=bool)
    for r in range(NN):
        for k in range(K):
            if not edge_mask[r, k]:
                continue
            s = int(in_idx[r, k])
            if s < NUM_IN or (0 <= write_pos[s] < r):
                valid[r, k] = True

    # prune nodes that do not reach any output
    needed = np.zeros(N, dtype=bool)
    needed[output_idx] = True
    for r in range(NN - 1, -1, -1):
        d = out_idx[r]
        if needed[d]:
            for k in range(K):
                if valid[r, k]:
                    needed[in_idx[r, k]] = True

    # topological levels over reachable non-input nodes (inputs = level 0)
    level = np.zeros(N, dtype=np.int64)
    for r in range(NN):
        d = out_idx[r]
        if not needed[d]:
            continue
        lmax = 0
        for k in range(K):
            if valid[r, k]:
                lmax = max(lmax, level[in_idx[r, k]] + 1)
        level[d] = lmax
    depth = int(level[needed].max()) if needed.any() else 0

    # split any level wider than 128 (keeps chunk packing valid)
    groups = []  # arrays of node ids, dependency order
    for l in range(1, depth + 1):
        nodes = [out_idx[r] for r in range(NN)
                 if needed[out_idx[r]] and level[out_idx[r]] == l]
        nodes = np.array(sorted(nodes, key=lambda d: write_pos[d]), dtype=np.int64)
        for i in range(0, len(nodes), 128):
            groups.append(nodes[i:i + 128])

    # pack whole groups into 128-row node chunks
    chunks = []   # list of list[(group_nodes, local_start)]
    fill = 128
    for g in groups:
        if fill + len(g) > 128:
            chunks.append([])
            fill = 0
        chunks[-1].append((g, fill))
        fill += len(g)

    # re-layout the last chunk so its activation slices leave already-final
    # output rows untouched: level 1 starts at the next 32-boundary after
    # level 0, and the final level gets a fresh 32-quadrant of its own
    last = chunks[-1]
    if len(last) > 1:
        starts = [0]
        end = len(last[0][0])
        for i in range(1, len(last)):
            if i == 1 or i == len(last) - 1:
                ns = max(end, 32 * ((end + 31) // 32))
            else:
                ns = end
            starts.append(ns)
            end = ns + len(last[i][0])
        if end <= 128:
            chunks[-1] = [(g, ns) for (g, _), ns in zip(last, starts)]

    n_in_chunks = NUM_IN // 128          # 2
    n_node_chunks = len(chunks)
    n_chunks = n_in_chunks + n_node_chunks

    chunk_of = np.full(N, -1, dtype=np.int64)
    row_of = np.full(N, -1, dtype=np.int64)
    for j in range(NUM_IN):
        chunk_of[j] = j // 128
        row_of[j] = j % 128
    for ci, levs in enumerate(chunks):
        for g, start in levs:
            for i, d in enumerate(g):
                chunk_of[d] = n_in_chunks + ci
                row_of[d] = start + i

    # per-node bias/response laid out per chunk
    bias_c = np.zeros((128, n_node_chunks), dtype=np.float32)
    resp_c = np.ones((128, n_node_chunks), dtype=np.float32)
    for r in range(NN):
        d = out_idx[r]
        if not needed[d]:
            continue
        bias_c[row_of[d], chunk_of[d] - n_in_chunks] = bias[r]
        resp_c[row_of[d], chunk_of[d] - n_in_chunks] = response[r]

    # weight blocks
    wa_blocks = {}   # (dst_chunk_rel, src_chunk_abs) -> [128,128]
    wb_blocks = {}   # (dst_chunk_rel, group_idx_in_chunk) -> [128,128]
    for r in range(NN):
        d = out_idx[r]
        if not needed[d]:
            continue
        dc = chunk_of[d] - n_in_chunks
        for k in range(K):
            if not valid[r, k]:
                continue
            s = int(in_idx[r, k])
            w = float(edge_w[r, k])
            sc = chunk_of[s]
            if sc == chunk_of[d]:
                gi = next(i for i, (g, st) in enumerate(chunks[dc])
                          if st <= row_of[d] < st + len(g))
                blk = wb_blocks.setdefault((dc, gi), np.zeros((128, 128), np.float32))
            else:
                blk = wa_blocks.setdefault((dc, sc), np.zeros((128, 128), np.float32))
            blk[row_of[s], row_of[d]] += w

    # input digest: when a late chunk reads <=128 distinct input nodes,
    # the host gathers those x columns into one extra v tile so the chunk
    # needs a single input matmul block instead of two
    digest_feeders = None
    dig = n_node_chunks - 1
    if (dig, 0) in wa_blocks and (dig, 1) in wa_blocks:
        b0, b1 = wa_blocks[(dig, 0)], wa_blocks[(dig, 1)]
        f0 = np.where((b0 != 0).any(axis=1))[0]
        f1 = np.where((b1 != 0).any(axis=1))[0]
        if len(f0) + len(f1) <= 128:
            digest_feeders = np.concatenate([f0, 128 + f1]).astype(np.int64)
            wg = np.zeros((128, 128), np.float32)
            wg[0:len(f0)] = b0[f0]
            wg[len(f0):len(f0) + len(f1)] = b1[f1]
            del wa_blocks[(dig, 0)], wa_blocks[(dig, 1)]
            wa_blocks[(dig, -1)] = wg      # src -1 = digest pseudo-chunk

    parta = []   # per node chunk: list of (src_chunk, wa_index), src ascending
    wa_list = []
    for dc in range(n_node_chunks):
        lst = []
        for sc in [-1] + list(range(n_chunks)):
            if (dc, sc) in wa_blocks:
                lst.append((sc, len(wa_list)))
                wa_list.append(wa_blocks[(dc, sc)])
        parta.append(lst)

    partb = []   # per node chunk: list of (group_idx, local_start, m, wb_index|None)
    wb_list = []
    for dc in range(n_node_chunks):
        lst = []
        for gi, (g, st) in enumerate(chunks[dc]):
            if (dc, gi) in wb_blocks:
                lst.append((gi, st, len(g), len(wb_list)))
                wb_list.append(wb_blocks[(dc, gi)])
            else:
                lst.append((gi, st, len(g), None))
        partb.append(lst)

    # output extraction: row-ranges per chunk covering its output nodes
    rows_by_chunk = {}
    for d in output_idx:
        dc = int(chunk_of[d]) - n_in_chunks
        rows_by_chunk.setdefault(dc, []).append(int(row_of[d]))
    out_ranges = []   # (dst_chunk_rel, row0, len, col0)
    col = 0
    pos_of = {}       # (dc, row) -> staged row in o16
    last_oc = max(rows_by_chunk)
    for dc in sorted(rows_by_chunk):
        rows = sorted(set(rows_by_chunk[dc]))
        gap = 10**9 if dc == last_oc else 8
        start = prev = rows[0]
        for r in rows[1:] + [None]:
            if r is not None and r - prev <= gap:
                prev = r
                continue
            ln = prev - start + 1
            out_ranges.append((dc, start, ln, col))
            for rr in range(start, prev + 1):
                pos_of[(dc, rr)] = col + rr - start
            col += ln
            if r is not None:
                start = prev = r
    ntot = col
    assert ntot <= 192, f"staged output rows {ntot} too large"
    colmap = np.zeros(NUM_OUT, dtype=np.int64)
    for oc, d in enumerate(output_idx):
        dc = int(chunk_of[d]) - n_in_chunks
        colmap[oc] = pos_of[(dc, int(row_of[d]))]

    # host-packed weight images: [128, n*128] partition-major so one DMA row
    # per partition is contiguous in DRAM
    def pack(blocks):
        if not blocks:
            return np.zeros((128, 128), np.float16)
        arr = np.stack(blocks).astype(np.float16)        # [n, 128, 128]
        return np.ascontiguousarray(arr.transpose(1, 0, 2).reshape(128, -1))

    return dict(
        out_ranges=out_ranges,
        ntot=ntot,
        colmap=colmap,
        digest_feeders=digest_feeders,
        n_in_chunks=n_in_chunks,
        n_node_chunks=n_node_chunks,
        n_chunks=n_chunks,
        parta=parta,
        partb=partb,
        n_wa=len(wa_list),
        n_wb=len(wb_list),
        wa_pk=pack(wa_list),
        wb_pk=pack(wb_list),
        bias_c=bias_c,
        resp_c=resp_c,
    )


def _build_nc(plan):
    n_in_chunks = plan["n_in_chunks"]
    n_node_chunks = plan["n_node_chunks"]
    n_chunks = plan["n_chunks"]
    parta = plan["parta"]
    partb = plan["partb"]
    out_ranges = plan["out_ranges"]
    ntot = plan["ntot"]
    n_wa = max(plan["n_wa"], 1)
    n_wb = max(plan["n_wb"], 1)

    nc = bacc.Bacc()
    x = nc.dram_tensor("x", [128, 2 * BC], F16, kind="ExternalInput")
    wa = nc.dram_tensor("wa", [128, n_wa * 128], F16, kind="ExternalInput")
    wb = nc.dram_tensor("wb", [128, n_wb * 128], F16, kind="ExternalInput")
    br_d = nc.dram_tensor("br_c", [128, 2 * n_node_chunks], F32, kind="ExternalInput")
    o = nc.dram_tensor("o", [ntot, BC], F16, kind="ExternalOutput")
    feeders = plan["digest_feeders"]
    xg = (nc.dram_tensor("xg", [128, BC], F16, kind="ExternalInput")
          if feeders is not None else None)

    with TileContext(nc) as tc:
        with tc.tile_pool(name="const", bufs=1) as const, \
             tc.tile_pool(name="vpool", bufs=n_chunks) as vpool, \
             tc.tile_pool(name="pc", bufs=4, space="PSUM") as pcp:

            br_sb = const.tile([128, 2 * n_node_chunks], F32, tag="br")
            bias_sb = br_sb[:, 0:n_node_chunks]
            resp_sb = br_sb[:, n_node_chunks:2 * n_node_chunks]
            wa_sb = const.tile([128, n_wa * 128], F16, tag="wa_sb")
            wb_sb = const.tile([128, n_wb * 128], F16, tag="wb_sb")
            zt = const.tile([1, 2], F32, tag="zt")
            wz = const.tile([128, 512], F16, tag="wz")

            # node-major value store: one [128, 4096] tile holds both input
            # chunks (in0|in1 x passA|passB column blocks, host-packed);
            # node chunks are [128, 2048] tiles, plus an optional digest
            vin = vpool.tile([128, 2 * BC], F16, tag="v", name="vin")
            v = {}
            for c in range(n_in_chunks, n_chunks):
                v[c] = vpool.tile([128, BC], F16, tag="v", name=f"v{c}")
            if feeders is not None:
                v[n_chunks] = vpool.tile([128, BC], F16, tag="v", name="vdig")

            def mov(sc, P, q):
                if sc < n_in_chunks:
                    c0 = P * 2 * HALF + sc * HALF + q * 512
                    return vin[:, c0:c0 + 512]
                return v[sc][:, P * HALF + q * 512:P * HALF + (q + 1) * 512]

            # warm-up constants first (gates the PE ramp; memset on the Pool
            # engine so the idle DVE preamble doesn't delay it), then trigger
            # the tanh table load on the scalar engine; bias passed as an AP
            # so no const-pool DMA sits on the dummy act's critical path
            nc.gpsimd.memset(wz[:], 0.0)
            nc.vector.memset(zt[:], 0.0)
            nc.scalar.activation(zt[0:1, 1:2], zt[0:1, 0:1],
                                 mybir.ActivationFunctionType.Tanh,
                                 bias=zt[0:1, 0:1])

            # used-row extent per chunk: matmul contraction is sliced to it,
            # so rows the acts never write are never read either
            csize = [128] * n_in_chunks
            for dc in range(n_node_chunks):
                gi, st, m, bi = partb[dc][-1]
                csize.append(min(((st + m + 31) // 32) * 32, 128))
            if feeders is not None:
                csize.append(min(((len(feeders) + 31) // 32) * 32, 128))

            # ---- DMA schedule, urgency-ordered: chunk-0 weights first, then
            # pass-A input (one DMA per input chunk), bias, chunk-1 weights,
            # pass-B input, remaining weights.
            def dma_x(half):
                nc.sync.dma_start(
                    vin[:, half * BC:(half + 1) * BC],
                    x[:, half * BC:(half + 1) * BC])

            def dma_x_q(half, q):
                b = half * BC + q * 512
                nc.sync.dma_start(
                    vin.rearrange("p (c f) -> p c f", f=512)[:, b // 512::2, :][:, 0:2, :],
                    x.rearrange("p (c f) -> p c f", f=512)[:, b // 512::2, :][:, 0:2, :])

            def wa_span(dc):
                idxs = [ai for _, ai in parta[dc]]
                return (idxs[0], idxs[-1] + 1) if idxs else None

            def wb_span(dc):
                bidx = [bi for _, _, _, bi in partb[dc] if bi is not None]
                return (bidx[0], bidx[-1] + 1) if bidx else None

            def dma_wa(i0, i1):
                nc.sync.dma_start(wa_sb[:, i0 * 128:i1 * 128],
                                  wa[:, i0 * 128:i1 * 128])

            def dma_wb(i0, i1):
                nc.sync.dma_start(wb_sb[:, i0 * 128:i1 * 128],
                                  wb[:, i0 * 128:i1 * 128])

            # pool/SWDGE queue (bypasses the shared HWDGE device): bias and
            # chunk-0 intra weights
            nc.gpsimd.dma_start(br_sb[:], br_d[:])
            s = wb_span(0)
            if s:
                nc.gpsimd.dma_start(wb_sb[:, s[0] * 128:s[1] * 128],
                                    wb[:, s[0] * 128:s[1] * 128])
            # sync/HWDGE queue, urgency-ordered
            dma_x(0)
            s = wa_span(0)
            if s:
                dma_wa(*s)
            dma_x(1)
            s0 = wa_span(1)
            if s0:
                dma_wa(*s0)
            s1 = wb_span(1)
            if s1:
                dma_wb(*s1)
            # remaining weights in pieces
            a_lo = wa_span(2)[0] if n_node_chunks > 2 and wa_span(2) else plan["n_wa"]
            if a_lo < plan["n_wa"]:
                mid = (a_lo + plan["n_wa"] + 1) // 2
                dma_wa(a_lo, mid)
                dma_wa(mid, plan["n_wa"])
            b_lo = wb_span(2)[0] if n_node_chunks > 2 and wb_span(2) else plan["n_wb"]
            if b_lo < plan["n_wb"]:
                dma_wb(b_lo, plan["n_wb"])
            if feeders is not None:
                for h in range(2):
                    nc.sync.dma_start(
                        v[n_chunks][:, h * HALF:(h + 1) * HALF],
                        xg[:, h * HALF:(h + 1) * HALF])

            # ---- PE warm-up: zero matmuls ramp the tensor-engine p-state
            # during the input DMA so the first real matmuls run full speed.
            # psum buffers are managed manually: (stream, chunk) -> buffer
            # (2*chunk+stream) % 4, so a stream's chunk c+2 can start
            # accumulating as soon as its chunk c is fully activated.
            pc_bufs = [pcp.tile([128, HALF], F32, tag="pc", name=f"pcb{i}")
                       for i in range(4)]

            def get_pc(P, dc):
                return pc_bufs[(2 * dc + P) % 4]

            for _ in range(7):
                nc.tensor.matmul(pc_bufs[0][:, 0:512], wz[:, 0:128], wz[:, :],
                                 start=True, stop=False, skip_group_check=True)

            # ---- cascade: the two batch passes (streams A=0, B=1) are
            # zipped at LEVEL granularity over the same chunk, B lagging one
            # level: A_L0, A_L1, B_L0, A_L2, B_L1, ..., B_L(last).  The act
            # queue is in-order, so alternating independent streams hides
            # each act's sem+matmul dependency path (~780ns) under the other
            # stream's 1038ns activation, keeping the scalar engine at full
            # cadence even when dependency-bound.
            started = {}                   # (P, dc, q) -> bool

            # filler units: one 512-wide cross-chunk matmul each.
            # ready_c = chunk pair after which the source values exist.
            units = []                     # [ready_c, dst_c, P, sc, ai, q]
            for dc in range(n_node_chunks):
                for P in range(2):
                    for sc, ai in parta[dc]:
                        ready = 0 if sc < n_in_chunks else sc - n_in_chunks + 1
                        for q in range(NQ):
                            units.append((ready, dc, P, sc, ai, q))
            units = [(r, d, P, (n_chunks if sc == -1 else sc), ai, q)
                     for r, d, P, sc, ai, q in units]
            units.sort(key=lambda u: (u[1], u[2], u[5], u[3]))
            emitted = [False] * len(units)
            acts_done = [[False] * n_node_chunks for _ in range(2)]

            def emit_unit(i):
                ready, dcu, P, sc, ai, q = units[i]
                pcu = get_pc(P, dcu)
                ks = csize[sc]
                nc.tensor.matmul(
                    pcu[:, q * 512:(q + 1) * 512],
                    wa_sb[0:ks, ai * 128:(ai + 1) * 128],
                    mov(sc, P, q)[0:ks, :],
                    start=not started.get((P, dcu, q), False), stop=False,
                    skip_group_check=True)
                started[(P, dcu, q)] = True
                emitted[i] = True

            def src_ready(i):
                r = units[i][0]
                return r == 0 or acts_done[units[i][2]][r - 1]

            def act_slice(dc_, gi_):
                gi, st, m, bi = partb[dc_][gi_]
                al = (st // 32) * 32
                if st + m <= al + 32:
                    return al, min(al + 32, 128)
                return 0, min(((st + m + 31) // 32) * 32, 128)

            def emit_fillers(cur_c, budget):
                n = 0
                for i in range(len(units)):
                    if n >= budget:
                        break
                    if emitted[i] or not src_ready(i):
                        continue
                    dst, P = units[i][1], units[i][2]
                    # psum buffer for (P, dst) conflicts with (P, dst-2):
                    # that chunk's acts must all be emitted first
                    if dst > cur_c + 1 and not (dst == cur_c + 2
                                                and acts_done[P][dst - 2]):
                        continue
                    emit_unit(i)
                    n += 1

            def emit_act(P, dc, gi_idx):
                gc = n_in_chunks + dc
                gi, st, m, bi = partb[dc][gi_idx]
                pcu = get_pc(P, dc)
                # fillers first: they sit BEFORE this slot's deadline-critical
                # wb/forced matmuls in the in-order PE queue
                emit_fillers(dc, 3)
                if gi_idx == 0:
                    # all cross-chunk contributions must be in
                    for i in range(len(units)):
                        if not emitted[i] and units[i][1] == dc and units[i][2] == P:
                            emit_unit(i)
                if bi is not None:
                    # only rows written by earlier levels of this chunk are
                    # readable (act slices cover [0, prefix-max) contiguously)
                    ks = max(act_slice(dc, j)[1] for j in range(gi_idx))
                    wt = wb_sb[0:ks, bi * 128:(bi + 1) * 128]
                    for q in range(NQ):
                        nc.tensor.matmul(
                            pcu[:, q * 512:(q + 1) * 512],
                            wt,
                            v[gc][0:ks, P * HALF + q * 512:P * HALF + (q + 1) * 512],
                            start=False, stop=False,
                            skip_group_check=True)
                # per-level tanh: PSUM partition access must start 32-aligned
                # and stay within one 32-quadrant (unless starting at row 0),
                # so activate the level's full quadrant, or [0, end) rounded
                # up when it spans quadrants.  Extra rows recompute earlier
                # values or write garbage that later levels overwrite.
                # L0 acts of late chunks are quarter-split so the first half
                # can start before the full transition matmul burst finishes.
                lo_r, hi_r = act_slice(dc, gi_idx)
                if gi_idx == 0 and dc >= 4:
                    for q in range(NQ):
                        nc.scalar.activation(
                            v[gc][lo_r:hi_r,
                                  P * HALF + q * 512:P * HALF + (q + 1) * 512],
                            pcu[lo_r:hi_r, q * 512:(q + 1) * 512],
                            mybir.ActivationFunctionType.Tanh,
                            bias=bias_sb[lo_r:hi_r, dc:dc + 1],
                            scale=resp_sb[lo_r:hi_r, dc:dc + 1])
                else:
                    nc.scalar.activation(
                        v[gc][lo_r:hi_r, P * HALF:(P + 1) * HALF],
                        pcu[lo_r:hi_r, :],
                        mybir.ActivationFunctionType.Tanh,
                        bias=bias_sb[lo_r:hi_r, dc:dc + 1],
                        scale=resp_sb[lo_r:hi_r, dc:dc + 1])
                if gi_idx == len(partb[dc]) - 1:
                    acts_done[P][dc] = True
                # last chunk: stream final output rows once no later act
                # slice can rewrite them, keeping the post-last-act DMA tiny
                if dc == n_node_chunks - 1 and P == 1:
                    nlev = len(partb[dc])
                    flush_hi = 128 if gi_idx == nlev - 1 else min(
                        act_slice(dc, j)[0] for j in range(gi_idx + 1, nlev))
                    rgs = [(r0, ln, c0) for c, r0, ln, c0 in out_ranges
                           if c == dc]
                    for r0, ln, c0 in rgs:
                        lo = max(r0, oflush[0])
                        hi = min(r0 + ln, flush_hi)
                        if lo < hi:
                            nc.sync.dma_start(
                                o[c0 + lo - r0:c0 + hi - r0, :],
                                v[gc][lo:hi, :])
                    oflush[0] = max(oflush[0], flush_hi)

            oq = [0]
            oflush = [0]
            # global slot list: stream B staggered one act behind stream A,
            # so adjacent acts always come from independent streams and each
            # act's sem+matmul dependency path hides under the other stream
            a_seq = [(0, dc, i) for dc in range(n_node_chunks)
                     for i in range(len(partb[dc]))]
            b_seq = [(1, dc, i) for _, dc, i in a_seq]
            slots = [a_seq[0]]
            for i in range(1, len(a_seq)):
                slots.append(a_seq[i])
                slots.append(b_seq[i - 1])
            slots.append(b_seq[-1])
            assert len(slots) == 2 * len(a_seq)
            for P, dc, gi_idx in slots:
                emit_act(P, dc, gi_idx)
                # stream a finished chunk's output rows out
                if (P == 1 and gi_idx == len(partb[dc]) - 1
                        and dc < n_node_chunks - 1):
                    rgs = [(r0, ln, c0) for c, r0, ln, c0 in out_ranges
                           if c == dc]
                    for r0, ln, c0 in rgs:
                        eng = nc.sync if oq[0] % 2 == 0 else nc.scalar
                        oq[0] += 1
                        eng.dma_start(o[c0:c0 + ln, :],
                                      v[n_in_chunks + dc][r0:r0 + ln, :])

    nc.compile()
    return nc


_CACHE = {}


def _get_compiled(key, plan):
    if key not in _CACHE:
        _CACHE[key] = _build_nc(plan)
    return _CACHE[key]


def kernel(inputs, edge_w, bias, response, in_idx, edge_mask, out_idx, output_idx):
    inputs = np.ascontiguousarray(np.asarray(inputs, dtype=np.float32))
    plan = _plan(in_idx, edge_mask, edge_w, bias, response, out_idx, output_idx)

    key = (plan["wa_pk"].tobytes(), plan["wb_pk"].tobytes(),
           str(plan["out_ranges"]), plan["bias_c"].tobytes(),
           plan["resp_c"].tobytes())
    nc = _get_compiled(hash(key), plan)

    base = {
        "wa": plan["wa_pk"],
        "wb": plan["wb_pk"],
        "br_c": np.ascontiguousarray(
            np.concatenate([plan["bias_c"], plan["resp_c"]], axis=1)),
    }

    x16 = inputs.astype(np.float16)
    feeders = plan["digest_feeders"]
    in_maps = []
    for c in range(NCORES):
        m = dict(base)
        # node-major transpose packed as [128, in0A|in1A|in0B|in1B]
        xt = x16[c * BC:(c + 1) * BC].T
        xh = np.empty((128, 2 * BC), np.float16)
        xh[:, 0:HALF] = xt[0:128, 0:HALF]
        xh[:, HALF:2 * HALF] = xt[128:256, 0:HALF]
        xh[:, 2 * HALF:3 * HALF] = xt[0:128, HALF:2 * HALF]
        xh[:, 3 * HALF:4 * HALF] = xt[128:256, HALF:2 * HALF]
        m["x"] = xh
        if feeders is not None:
            g = np.zeros((128, BC), np.float16)
            g[0:len(feeders)] = xt[feeders]
            m["xg"] = g
        in_maps.append(m)

    res = run_bass_kernel_spmd(nc, in_maps, core_ids=list(range(NCORES)))
    kernel.last_results = res
    colmap = np.asarray(plan["colmap"])
    out = np.concatenate(
        [res.results[c]["o"][colmap].T for c in range(NCORES)], axis=0)
    return np.ascontiguousarray(out.astype(np.float32))


kernel.last_results = None


# revision 42
# speedup vs baseline: 1.0429x; 1.0429x over previous
"""Trainium2 Bass kernel for NEAT-style genome evaluation (gnn_message_passing).

Shapes are hardcoded for the problem:
  inputs [16384, 256] f32, in_idx/edge_w/edge_mask [768, 8], bias/response [768],
  out_idx [768] (scan order), output_idx [64]; output [16384, 64] f32.

Strategy: pure data-parallel over batch (2048 rows/core on 8 cores). The host
pre-transposes the input to node-major fp16 so DMA lands directly in the SBUF
value store v[chunk] = [128 nodes, 2048 batch] (no on-device marshaling).
Nodes are packed by topological level into 128-row chunks; per (level, pass)
the pre-activations accumulate in PSUM via fp16 matmuls whose [128,128]
stationary blocks embed the sparse DAG edges, and tanh(bias + response*s)
runs on the scalar engine over [128 rows, 1024 batch].  The batch is processed
in two sequential passes of 1024 columns so each chunk's PSUM tile is only
2 banks, allowing 4 live tiles and a 3-chunk cross-chunk matmul prefetch
horizon; cross-chunk ("parta") matmul quarters are greedily interleaved into
the level cascade as fillers so the tensor engine stays busy under the serial
activation chain.  Output node rows are DMA'd fp16 node-major; the host
gathers/transposes/converts.  fp16 keeps relative error ~3e-3 (tolerance 2e-2).
"""

import sys

import numpy as np

if "/opt/trn_rl_repo" not in sys.path:
    sys.path.insert(0, "/opt/trn_rl_repo")

import concourse.bacc as bacc
import concourse.mybir as mybir
from concourse.bass_utils import run_bass_kernel_spmd
from concourse.tile import TileContext

F16 = mybir.dt.float16
F32 = mybir.dt.float32

B = 16384
NUM_IN = 256
N = 1024
K = 8
NN = N - NUM_IN
NUM_OUT = 64
NCORES = 8
BC = B // NCORES          # batch rows per core (2048)
HALF = BC // 2            # batch columns per pass (1024)
NQ = 2                    # 512-wide matmul quarters per pass
HORIZON = 3               # psum prefetch horizon in chunks


def _plan(in_idx, edge_mask, edge_w, bias, response, out_idx, output_idx):
    """All host-side graph analysis; returns the constant tensors + schedule."""
    in_idx = np.asarray(in_idx)
    edge_mask = np.asarray(edge_mask).astype(bool)
    edge_w = np.asarray(edge_w).astype(np.float32)
    bias = np.asarray(bias).astype(np.float32)
    response = np.asarray(response).astype(np.float32)
    out_idx = np.asarray(out_idx)
    output_idx = np.asarray(output_idx)

    # scan write position of each node (reference writes out_idx[r] at step r)
    write_pos = np.full(N, -1, dtype=np.int64)
    for r in range(NN):
        write_pos[out_idx[r]] = r

    # valid edges: mask set AND source reads a value written before this step
    valid = np.zeros((NN, K), dtype=bool)
    for r in range(NN):
        for k in range(K):
            if not edge_mask[r, k]:
                continue
            s = int(in_idx[r, k])
            if s < NUM_IN or (0 <= write_pos[s] < r):
                valid[r, k] = True

    # prune nodes that do not reach any output
    needed = np.zeros(N, dtype=bool)
    needed[output_idx] = True
    for r in range(NN - 1, -1, -1):
        d = out_idx[r]
        if needed[d]:
            for k in range(K):
                if valid[r, k]:
                    needed[in_idx[r, k]] = True

    # topological levels over reachable non-input nodes (inputs = level 0)
    level = np.zeros(N, dtype=np.int64)
    for r in range(NN):
        d = out_idx[r]
        if not needed[d]:
            continue
        lmax = 0
        for k in range(K):
            if valid[r, k]:
                lmax = max(lmax, level[in_idx[r, k]] + 1)
        level[d] = lmax
    depth = int(level[needed].max()) if needed.any() else 0

    # split any level wider than 128 (keeps chunk packing valid)
    groups = []  # arrays of node ids, dependency order
    for l in range(1, depth + 1):
        nodes = [out_idx[r] for r in range(NN)
                 if needed[out_idx[r]] and level[out_idx[r]] == l]
        nodes = np.array(sorted(nodes, key=lambda d: write_pos[d]), dtype=np.int64)
        for i in range(0, len(nodes), 128):
            groups.append(nodes[i:i + 128])

    # pack whole groups into 128-row node chunks
    chunks = []   # list of list[(group_nodes, local_start)]
    fill = 128
    for g in groups:
        if fill + len(g) > 128:
            chunks.append([])
            fill = 0
        chunks[-1].append((g, fill))
        fill += len(g)

    # re-layout the last chunk so its activation slices leave already-final
    # output rows untouched: level 1 starts at the next 32-boundary after
    # level 0, and the final level gets a fresh 32-quadrant of its own
    last = chunks[-1]
    if len(last) > 1:
        starts = [0]
        end = len(last[0][0])
        for i in range(1, len(last)):
            if i == 1 or i == len(last) - 1:
                ns = max(end, 32 * ((end + 31) // 32))
            else:
                ns = end
            starts.append(ns)
            end = ns + len(last[i][0])
        if end <= 128:
            chunks[-1] = [(g, ns) for (g, _), ns in zip(last, starts)]

    n_in_chunks = NUM_IN // 128          # 2
    n_node_chunks = len(chunks)
    n_chunks = n_in_chunks + n_node_chunks

    chunk_of = np.full(N, -1, dtype=np.int64)
    row_of = np.full(N, -1, dtype=np.int64)
    for j in range(NUM_IN):
        chunk_of[j] = j // 128
        row_of[j] = j % 128
    for ci, levs in enumerate(chunks):
        for g, start in levs:
            for i, d in enumerate(g):
                chunk_of[d] = n_in_chunks + ci
                row_of[d] = start + i

    # per-node bias/response laid out per chunk
    bias_c = np.zeros((128, n_node_chunks), dtype=np.float32)
    resp_c = np.ones((128, n_node_chunks), dtype=np.float32)
    for r in range(NN):
        d = out_idx[r]
        if not needed[d]:
            continue
        bias_c[row_of[d], chunk_of[d] - n_in_chunks] = bias[r]
        resp_c[row_of[d], chunk_of[d] - n_in_chunks] = response[r]

    # weight blocks
    wa_blocks = {}   # (dst_chunk_rel, src_chunk_abs) -> [128,128]
    wb_blocks = {}   # (dst_chunk_rel, group_idx_in_chunk) -> [128,128]
    for r in range(NN):
        d = out_idx[r]
        if not needed[d]:
            continue
        dc = chunk_of[d] - n_in_chunks
        for k in range(K):
            if not valid[r, k]:
                continue
            s = int(in_idx[r, k])
            w = float(edge_w[r, k])
            sc = chunk_of[s]
            if sc == chunk_of[d]:
                gi = next(i for i, (g, st) in enumerate(chunks[dc])
                          if st <= row_of[d] < st + len(g))
                blk = wb_blocks.setdefault((dc, gi), np.zeros((128, 128), np.float32))
            else:
                blk = wa_blocks.setdefault((dc, sc), np.zeros((128, 128), np.float32))
            blk[row_of[s], row_of[d]] += w

    # input digest: when a late chunk reads <=128 distinct input nodes,
    # the host gathers those x columns into one extra v tile so the chunk
    # needs a single input matmul block instead of two
    digest_feeders = None
    dig = n_node_chunks - 1
    if (dig, 0) in wa_blocks and (dig, 1) in wa_blocks:
        b0, b1 = wa_blocks[(dig, 0)], wa_blocks[(dig, 1)]
        f0 = np.where((b0 != 0).any(axis=1))[0]
        f1 = np.where((b1 != 0).any(axis=1))[0]
        if len(f0) + len(f1) <= 128:
            digest_feeders = np.concatenate([f0, 128 + f1]).astype(np.int64)
            wg = np.zeros((128, 128), np.float32)
            wg[0:len(f0)] = b0[f0]
            wg[len(f0):len(f0) + len(f1)] = b1[f1]
            del wa_blocks[(dig, 0)], wa_blocks[(dig, 1)]
            wa_blocks[(dig, -1)] = wg      # src -1 = digest pseudo-chunk

    parta = []   # per node chunk: list of (src_chunk, wa_index), src ascending
    wa_list = []
    for dc in range(n_node_chunks):
        lst = []
        for sc in [-1] + list(range(n_chunks)):
            if (dc, sc) in wa_blocks:
                lst.append((sc, len(wa_list)))
                wa_list.append(wa_blocks[(dc, sc)])
        parta.append(lst)

    partb = []   # per node chunk: list of (group_idx, local_start, m, wb_index|None)
    wb_list = []
    for dc in range(n_node_chunks):
        lst = []
        for gi, (g, st) in enumerate(chunks[dc]):
            if (dc, gi) in wb_blocks:
                lst.append((gi, st, len(g), len(wb_list)))
                wb_list.append(wb_blocks[(dc, gi)])
            else:
                lst.append((gi, st, len(g), None))
        partb.append(lst)

    # output extraction: row-ranges per chunk covering its output nodes
    rows_by_chunk = {}
    for d in output_idx:
        dc = int(chunk_of[d]) - n_in_chunks
        rows_by_chunk.setdefault(dc, []).append(int(row_of[d]))
    out_ranges = []   # (dst_chunk_rel, row0, len, col0)
    col = 0
    pos_of = {}       # (dc, row) -> staged row in o16
    last_oc = max(rows_by_chunk)
    for dc in sorted(rows_by_chunk):
        rows = sorted(set(rows_by_chunk[dc]))
        gap = 10**9 if dc == last_oc else 8
        start = prev = rows[0]
        for r in rows[1:] + [None]:
            if r is not None and r - prev <= gap:
                prev = r
                continue
            ln = prev - start + 1
            out_ranges.append((dc, start, ln, col))
            for rr in range(start, prev + 1):
                pos_of[(dc, rr)] = col + rr - start
            col += ln
            if r is not None:
                start = prev = r
    ntot = col
    assert ntot <= 192, f"staged output rows {ntot} too large"
    colmap = np.zeros(NUM_OUT, dtype=np.int64)
    for oc, d in enumerate(output_idx):
        dc = int(chunk_of[d]) - n_in_chunks
        colmap[oc] = pos_of[(dc, int(row_of[d]))]

    # host-packed weight images: [128, n*128] partition-major so one DMA row
    # per partition is contiguous in DRAM
    def pack(blocks):
        if not blocks:
            return np.zeros((128, 128), np.float16)
        arr = np.stack(blocks).astype(np.float16)        # [n, 128, 128]
        return np.ascontiguousarray(arr.transpose(1, 0, 2).reshape(128, -1))

    return dict(
        out_ranges=out_ranges,
        ntot=ntot,
        colmap=colmap,
        digest_feeders=digest_feeders,
        n_in_chunks=n_in_chunks,
        n_node_chunks=n_node_chunks,
        n_chunks=n_chunks,
        parta=parta,
        partb=partb,
        n_wa=len(wa_list),
        n_wb=len(wb_list),
        wa_pk=pack(wa_list),
        wb_pk=pack(wb_list),
        bias_c=bias_c,
        resp_c=resp_c,
    )


def _build_nc(plan):
    n_in_chunks = plan["n_in_chunks"]
    n_node_chunks = plan["n_node_chunks"]
    n_chunks = plan["n_chunks"]
    parta = plan["parta"]
    partb = plan["partb"]
    out_ranges = plan["out_ranges"]
    ntot = plan["ntot"]
    n_wa = max(plan["n_wa"], 1)
    n_wb = max(plan["n_wb"], 1)

    nc = bacc.Bacc()
    x = nc.dram_tensor("x", [128, 2 * BC], F16, kind="ExternalInput")
    wa = nc.dram_tensor("wa", [128, n_wa * 128], F16, kind="ExternalInput")
    wb = nc.dram_tensor("wb", [128, n_wb * 128], F16, kind="ExternalInput")
    br_d = nc.dram_tensor("br_c", [128, 2 * n_node_chunks], F32, kind="ExternalInput")
    o = nc.dram_tensor("o", [ntot, BC], F16, kind="ExternalOutput")
    feeders = plan["digest_feeders"]
    xg = (nc.dram_tensor("xg", [128, BC], F16, kind="ExternalInput")
          if feeders is not None else None)

    with TileContext(nc) as tc:
        with tc.tile_pool(name="const", bufs=1) as const, \
             tc.tile_pool(name="vpool", bufs=n_chunks) as vpool, \
             tc.tile_pool(name="pc", bufs=4, space="PSUM") as pcp:

            br_sb = const.tile([128, 2 * n_node_chunks], F32, tag="br")
            bias_sb = br_sb[:, 0:n_node_chunks]
            resp_sb = br_sb[:, n_node_chunks:2 * n_node_chunks]
            wa_sb = const.tile([128, n_wa * 128], F16, tag="wa_sb")
            wb_sb = const.tile([128, n_wb * 128], F16, tag="wb_sb")
            zt = const.tile([1, 2], F32, tag="zt")
            wz = const.tile([128, 512], F16, tag="wz")

            # node-major value store: one [128, 4096] tile holds both input
            # chunks (in0|in1 x passA|passB column blocks, host-packed);
            # node chunks are [128, 2048] tiles, plus an optional digest
            vin = vpool.tile([128, 2 * BC], F16, tag="v", name="vin")
            v = {}
            for c in range(n_in_chunks, n_chunks):
                v[c] = vpool.tile([128, BC], F16, tag="v", name=f"v{c}")
            if feeders is not None:
                v[n_chunks] = vpool.tile([128, BC], F16, tag="v", name="vdig")

            def mov(sc, P, q):
                if sc < n_in_chunks:
                    c0 = P * 2 * HALF + sc * HALF + q * 512
                    return vin[:, c0:c0 + 512]
                return v[sc][:, P * HALF + q * 512:P * HALF + (q + 1) * 512]

            # warm-up constants first (gates the PE ramp; memset on the Pool
            # engine so the idle DVE preamble doesn't delay it), then trigger
            # the tanh table load on the scalar engine; bias passed as an AP
            # so no const-pool DMA sits on the dummy act's critical path
            nc.gpsimd.memset(wz[:], 0.0)
            nc.vector.memset(zt[:], 0.0)
            nc.scalar.activation(zt[0:1, 1:2], zt[0:1, 0:1],
                                 mybir.ActivationFunctionType.Tanh,
                                 bias=zt[0:1, 0:1])

            # used-row extent per chunk: matmul contraction is sliced to it,
            # so rows the acts never write are never read either
            csize = [128] * n_in_chunks
            for dc in range(n_node_chunks):
                gi, st, m, bi = partb[dc][-1]
                csize.append(min(((st + m + 31) // 32) * 32, 128))
            if feeders is not None:
                csize.append(min(((len(feeders) + 31) // 32) * 32, 128))

            # ---- DMA schedule, urgency-ordered: chunk-0 weights first, then
            # pass-A input (one DMA per input chunk), bias, chunk-1 weights,
            # pass-B input, remaining weights.
            def dma_x(half):
                nc.sync.dma_start(
                    vin[:, half * BC:(half + 1) * BC],
                    x[:, half * BC:(half + 1) * BC])

            def dma_x_q(half, q):
                b = half * BC + q * 512
                nc.sync.dma_start(
                    vin.rearrange("p (c f) -> p c f", f=512)[:, b // 512::2, :][:, 0:2, :],
                    x.rearrange("p (c f) -> p c f", f=512)[:, b // 512::2, :][:, 0:2, :])

            def wa_span(dc):
                idxs = [ai for _, ai in parta[dc]]
                return (idxs[0], idxs[-1] + 1) if idxs else None

            def wb_span(dc):
                bidx = [bi for _, _, _, bi in partb[dc] if bi is not None]
                return (bidx[0], bidx[-1] + 1) if bidx else None

            def dma_wa(i0, i1):
                nc.sync.dma_start(wa_sb[:, i0 * 128:i1 * 128],
                                  wa[:, i0 * 128:i1 * 128])

            def dma_wb(i0, i1):
                nc.sync.dma_start(wb_sb[:, i0 * 128:i1 * 128],
                                  wb[:, i0 * 128:i1 * 128])

            # pool/SWDGE queue (bypasses the shared HWDGE device): bias and
            # chunk-0 intra weights
            nc.gpsimd.dma_start(br_sb[:], br_d[:])
            s = wb_span(0)
            if s:
                nc.gpsimd.dma_start(wb_sb[:, s[0] * 128:s[1] * 128],
                                    wb[:, s[0] * 128:s[1] * 128])
            # sync/HWDGE queue, urgency-ordered
            dma_x(0)
            s = wa_span(0)
            if s:
                dma_wa(*s)
            dma_x(1)
            s0 = wa_span(1)
            if s0:
                dma_wa(*s0)
            s1 = wb_span(1)
            if s1:
                dma_wb(*s1)
            # remaining weights in pieces
            a_lo = wa_span(2)[0] if n_node_chunks > 2 and wa_span(2) else plan["n_wa"]
            if a_lo < plan["n_wa"]:
                mid = (a_lo + plan["n_wa"] + 1) // 2
                dma_wa(a_lo, mid)
                dma_wa(mid, plan["n_wa"])
            b_lo = wb_span(2)[0] if n_node_chunks > 2 and wb_span(2) else plan["n_wb"]
            if b_lo < plan["n_wb"]:
                dma_wb(b_lo, plan["n_wb"])
            if feeders is not None:
                for h in range(2):
                    nc.sync.dma_start(
                        v[n_chunks][:, h * HALF:(h + 1) * HALF],
                        xg[:, h * HALF:(h + 1) * HALF])

            # ---- PE warm-up: zero matmuls ramp the tensor-engine p-state
            # during the input DMA so the first real matmuls run full speed.
            # psum buffers are managed manually: (stream, chunk) -> buffer
            # (2*chunk+stream) % 4, so a stream's chunk c+2 can start
            # accumulating as soon as its chunk c is fully activated.
            pc_bufs = [pcp.tile([128, HALF], F32, tag="pc", name=f"pcb{i}")
                       for i in range(4)]

            def get_pc(P, dc):
                return pc_bufs[(2 * dc + P) % 4]

            for _ in range(7):
                nc.tensor.matmul(pc_bufs[0][:, 0:512], wz[:, 0:128], wz[:, :],
                                 start=True, stop=False, skip_group_check=True)

            # ---- cascade: the two batch passes (streams A=0, B=1) are
            # zipped at LEVEL granularity over the same chunk, B lagging one
            # level: A_L0, A_L1, B_L0, A_L2, B_L1, ..., B_L(last).  The act
            # queue is in-order, so alternating independent streams hides
            # each act's sem+matmul dependency path (~780ns) under the other
            # stream's 1038ns activation, keeping the scalar engine at full
            # cadence even when dependency-bound.
            started = {}                   # (P, dc, q) -> bool

            # filler units: one 512-wide cross-chunk matmul each.
            # ready_c = chunk pair after which the source values exist.
            units = []                     # [ready_c, dst_c, P, sc, ai, q]
            for dc in range(n_node_chunks):
                for P in range(2):
                    for sc, ai in parta[dc]:
                        ready = 0 if sc < n_in_chunks else sc - n_in_chunks + 1
                        for q in range(NQ):
                            units.append((ready, dc, P, sc, ai, q))
            units = [(r, d, P, (n_chunks if sc == -1 else sc), ai, q)
                     for r, d, P, sc, ai, q in units]
            units.sort(key=lambda u: (u[1], u[2], u[5], u[3]))
            emitted = [False] * len(units)
            acts_done = [[False] * n_node_chunks for _ in range(2)]

            def emit_unit(i):
                ready, dcu, P, sc, ai, q = units[i]
                pcu = get_pc(P, dcu)
                ks = csize[sc]
                nc.tensor.matmul(
                    pcu[:, q * 512:(q + 1) * 512],
                    wa_sb[0:ks, ai * 128:(ai + 1) * 128],
                    mov(sc, P, q)[0:ks, :],
                    start=not started.get((P, dcu, q), False), stop=False,
                    skip_group_check=True)
                started[(P, dcu, q)] = True
                emitted[i] = True

            def src_ready(i):
                r = units[i][0]
                return r == 0 or acts_done[units[i][2]][r - 1]

            def act_slice(dc_, gi_):
                gi, st, m, bi = partb[dc_][gi_]
                al = (st // 32) * 32
                if st + m <= al + 32:
                    return al, min(al + 32, 128)
                return 0, min(((st + m + 31) // 32) * 32, 128)

            def emit_fillers(cur_c, budget):
                n = 0
                for i in range(len(units)):
                    if n >= budget:
                        break
                    if emitted[i] or not src_ready(i):
                        continue
                    dst, P = units[i][1], units[i][2]
                    # psum buffer for (P, dst) conflicts with (P, dst-2):
                    # that chunk's acts must all be emitted first
                    if dst > cur_c + 1 and not (dst == cur_c + 2
                                                and acts_done[P][dst - 2]):
                        continue
                    emit_unit(i)
                    n += 1

            def emit_act(P, dc, gi_idx):
                gc = n_in_chunks + dc
                gi, st, m, bi = partb[dc][gi_idx]
                pcu = get_pc(P, dc)
                # fillers first: they sit BEFORE this slot's deadline-critical
                # wb/forced matmuls in the in-order PE queue
                emit_fillers(dc, 3)
                if gi_idx == 0:
                    # all cross-chunk contributions must be in
                    for i in range(len(units)):
                        if not emitted[i] and units[i][1] == dc and units[i][2] == P:
                            emit_unit(i)
                if bi is not None:
                    # only rows written by earlier levels of this chunk are
                    # readable (act slices cover [0, prefix-max) contiguously)
                    ks = max(act_slice(dc, j)[1] for j in range(gi_idx))
                    wt = wb_sb[0:ks, bi * 128:(bi + 1) * 128]
                    for q in range(NQ):
                        nc.tensor.matmul(
                            pcu[:, q * 512:(q + 1) * 512],
                            wt,
                            v[gc][0:ks, P * HALF + q * 512:P * HALF + (q + 1) * 512],
                            start=False, stop=False,
                            skip_group_check=True)
                # per-level tanh: PSUM partition access must start 32-aligned
                # and stay within one 32-quadrant (unless starting at row 0),
                # so activate the level's full quadrant, or [0, end) rounded
                # up when it spans quadrants.  Extra rows recompute earlier
                # values or write garbage that later levels overwrite.
                # L0 acts of late chunks are quarter-split so the first half
                # can start before the full transition matmul burst finishes.
                lo_r, hi_r = act_slice(dc, gi_idx)
                if gi_idx == 0 and dc >= 4:
                    for q in range(NQ):
                        nc.scalar.activation(
                            v[gc][lo_r:hi_r,
                                  P * HALF + q * 512:P * HALF + (q + 1) * 512],
                            pcu[lo_r:hi_r, q * 512:(q + 1) * 512],
                            mybir.ActivationFunctionType.Tanh,
                            bias=bias_sb[lo_r:hi_r, dc:dc + 1],
                            scale=resp_sb[lo_r:hi_r, dc:dc + 1])
                else:
                    nc.scalar.activation(
                        v[gc][lo_r:hi_r, P * HALF:(P + 1) * HALF],
                        pcu[lo_r:hi_r, :],
                        mybir.ActivationFunctionType.Tanh,
                        bias=bias_sb[lo_r:hi_r, dc:dc + 1],
                        scale=resp_sb[lo_r:hi_r, dc:dc + 1])
                if gi_idx == len(partb[dc]) - 1:
                    acts_done[P][dc] = True
                # last chunk: stream final output rows once no later act
                # slice can rewrite them, keeping the post-last-act DMA tiny
                if dc == n_node_chunks - 1 and P == 1:
                    nlev = len(partb[dc])
                    flush_hi = 128 if gi_idx == nlev - 1 else min(
                        act_slice(dc, j)[0] for j in range(gi_idx + 1, nlev))
                    rgs = [(r0, ln, c0) for c, r0, ln, c0 in out_ranges
                           if c == dc]
                    for r0, ln, c0 in rgs:
                        lo = max(r0, oflush[0])
                        hi = min(r0 + ln, flush_hi)
                        if lo < hi:
                            nc.sync.dma_start(
                                o[c0 + lo - r0:c0 + hi - r0, :],
                                v[gc][lo:hi, :])
                    oflush[0] = max(oflush[0], flush_hi)

            oq = [0]
            oflush = [0]
            # global slot list: stream B staggered one act behind stream A,
            # so adjacent acts always come from independent streams and each
            # act's sem+matmul dependency path hides under the other stream
            slots = []
            for dc in range(n_node_chunks):
                for i in range(len(partb[dc])):
                    slots.append((0, dc, i))
                    slots.append((1, dc, i))
            for P, dc, gi_idx in slots:
                emit_act(P, dc, gi_idx)
                # stream a finished chunk's output rows out
                if (P == 1 and gi_idx == len(partb[dc]) - 1
                        and dc < n_node_chunks - 1):
                    rgs = [(r0, ln, c0) for c, r0, ln, c0 in out_ranges
                           if c == dc]
                    for r0, ln, c0 in rgs:
                        eng = nc.sync if oq[0] % 2 == 0 else nc.scalar
                        oq[0] += 1
                        eng.dma_start(o[c0:c0 + ln, :],
                                      v[n_in_chunks + dc][r0:r0 + ln, :])

    nc.compile()
    return nc


_CACHE = {}


def _get_compiled(key, plan):
    if key not in _CACHE:
        _CACHE[key] = _build_nc(plan)
    return _CACHE[key]


def kernel(inputs, edge_w, bias, response, in_idx, edge_mask, out_idx, output_idx):
    inputs = np.ascontiguousarray(np.asarray(inputs, dtype=np.float32))
    plan = _plan(in_idx, edge_mask, edge_w, bias, response, out_idx, output_idx)

    key = (plan["wa_pk"].tobytes(), plan["wb_pk"].tobytes(),
           str(plan["out_ranges"]), plan["bias_c"].tobytes(),
           plan["resp_c"].tobytes())
    nc = _get_compiled(hash(key), plan)

    base = {
        "wa": plan["wa_pk"],
        "wb": plan["wb_pk"],
        "br_c": np.ascontiguousarray(
            np.concatenate([plan["bias_c"], plan["resp_c"]], axis=1)),
    }

    x16 = inputs.astype(np.float16)
    feeders = plan["digest_feeders"]
    in_maps = []
    for c in range(NCORES):
        m = dict(base)
        # node-major transpose packed as [128, in0A|in1A|in0B|in1B]
        xt = x16[c * BC:(c + 1) * BC].T
        xh = np.empty((128, 2 * BC), np.float16)
        xh[:, 0:HALF] = xt[0:128, 0:HALF]
        xh[:, HALF:2 * HALF] = xt[128:256, 0:HALF]
        xh[:, 2 * HALF:3 * HALF] = xt[0:128, HALF:2 * HALF]
        xh[:, 3 * HALF:4 * HALF] = xt[128:256, HALF:2 * HALF]
        m["x"] = xh
        if feeders is not None:
            g = np.zeros((128, BC), np.float16)
            g[0:len(feeders)] = xt[feeders]
            m["xg"] = g
        in_maps.append(m)

    res = run_bass_kernel_spmd(nc, in_maps, core_ids=list(range(NCORES)))
    kernel.last_results = res
    colmap = np.asarray(plan["colmap"])
    out = np.concatenate(
        [res.results[c]["o"][colmap].T for c in range(NCORES)], axis=0)
    return np.ascontiguousarray(out.astype(np.float32))


kernel.last_results = None


# revision 43
# speedup vs baseline: 1.0819x; 1.0374x over previous
"""Trainium2 Bass kernel for NEAT-style genome evaluation (gnn_message_passing).

Shapes are hardcoded for the problem:
  inputs [16384, 256] f32, in_idx/edge_w/edge_mask [768, 8], bias/response [768],
  out_idx [768] (scan order), output_idx [64]; output [16384, 64] f32.

Strategy: pure data-parallel over batch (2048 rows/core on 8 cores). The host
pre-transposes the input to node-major fp16 so DMA lands directly in the SBUF
value store v[chunk] = [128 nodes, 2048 batch] (no on-device marshaling).
Nodes are packed by topological level into 128-row chunks; per (level, pass)
the pre-activations accumulate in PSUM via fp16 matmuls whose [128,128]
stationary blocks embed the sparse DAG edges, and tanh(bias + response*s)
runs on the scalar engine over [128 rows, 1024 batch].  The batch is processed
in two sequential passes of 1024 columns so each chunk's PSUM tile is only
2 banks, allowing 4 live tiles and a 3-chunk cross-chunk matmul prefetch
horizon; cross-chunk ("parta") matmul quarters are greedily interleaved into
the level cascade as fillers so the tensor engine stays busy under the serial
activation chain.  Output node rows are DMA'd fp16 node-major; the host
gathers/transposes/converts.  fp16 keeps relative error ~3e-3 (tolerance 2e-2).
"""

import sys

import numpy as np

if "/opt/trn_rl_repo" not in sys.path:
    sys.path.insert(0, "/opt/trn_rl_repo")

import concourse.bacc as bacc
import concourse.mybir as mybir
from concourse.bass_utils import run_bass_kernel_spmd
from concourse.tile import TileContext

F16 = mybir.dt.float16
F32 = mybir.dt.float32

B = 16384
NUM_IN = 256
N = 1024
K = 8
NN = N - NUM_IN
NUM_OUT = 64
NCORES = 8
BC = B // NCORES          # batch rows per core (2048)
HALF = BC // 2            # batch columns per pass (1024)
NQ = 2                    # 512-wide matmul quarters per pass
HORIZON = 3               # psum prefetch horizon in chunks


def _plan(in_idx, edge_mask, edge_w, bias, response, out_idx, output_idx):
    """All host-side graph analysis; returns the constant tensors + schedule."""
    in_idx = np.asarray(in_idx)
    edge_mask = np.asarray(edge_mask).astype(bool)
    edge_w = np.asarray(edge_w).astype(np.float32)
    bias = np.asarray(bias).astype(np.float32)
    response = np.asarray(response).astype(np.float32)
    out_idx = np.asarray(out_idx)
    output_idx = np.asarray(output_idx)

    # scan write position of each node (reference writes out_idx[r] at step r)
    write_pos = np.full(N, -1, dtype=np.int64)
    for r in range(NN):
        write_pos[out_idx[r]] = r

    # valid edges: mask set AND source reads a value written before this step
    valid = np.zeros((NN, K), dtype=bool)
    for r in range(NN):
        for k in range(K):
            if not edge_mask[r, k]:
                continue
            s = int(in_idx[r, k])
            if s < NUM_IN or (0 <= write_pos[s] < r):
                valid[r, k] = True

    # prune nodes that do not reach any output
    needed = np.zeros(N, dtype=bool)
    needed[output_idx] = True
    for r in range(NN - 1, -1, -1):
        d = out_idx[r]
        if needed[d]:
            for k in range(K):
                if valid[r, k]:
                    needed[in_idx[r, k]] = True

    # topological levels over reachable non-input nodes (inputs = level 0)
    level = np.zeros(N, dtype=np.int64)
    for r in range(NN):
        d = out_idx[r]
        if not needed[d]:
            continue
        lmax = 0
        for k in range(K):
            if valid[r, k]:
                lmax = max(lmax, level[in_idx[r, k]] + 1)
        level[d] = lmax
    depth = int(level[needed].max()) if needed.any() else 0

    # split any level wider than 128 (keeps chunk packing valid)
    groups = []  # arrays of node ids, dependency order
    for l in range(1, depth + 1):
        nodes = [out_idx[r] for r in range(NN)
                 if needed[out_idx[r]] and level[out_idx[r]] == l]
        nodes = np.array(sorted(nodes, key=lambda d: write_pos[d]), dtype=np.int64)
        for i in range(0, len(nodes), 128):
            groups.append(nodes[i:i + 128])

    # pack whole groups into 128-row node chunks
    chunks = []   # list of list[(group_nodes, local_start)]
    fill = 128
    for g in groups:
        if fill + len(g) > 128:
            chunks.append([])
            fill = 0
        chunks[-1].append((g, fill))
        fill += len(g)

    # re-layout the last chunk so its activation slices leave already-final
    # output rows untouched: level 1 starts at the next 32-boundary after
    # level 0, and the final level gets a fresh 32-quadrant of its own
    last = chunks[-1]
    if len(last) > 1:
        starts = [0]
        end = len(last[0][0])
        for i in range(1, len(last)):
            if i == 1 or i == len(last) - 1:
                ns = max(end, 32 * ((end + 31) // 32))
            else:
                ns = end
            starts.append(ns)
            end = ns + len(last[i][0])
        if end <= 128:
            chunks[-1] = [(g, ns) for (g, _), ns in zip(last, starts)]

    n_in_chunks = NUM_IN // 128          # 2
    n_node_chunks = len(chunks)
    n_chunks = n_in_chunks + n_node_chunks

    chunk_of = np.full(N, -1, dtype=np.int64)
    row_of = np.full(N, -1, dtype=np.int64)
    for j in range(NUM_IN):
        chunk_of[j] = j // 128
        row_of[j] = j % 128
    for ci, levs in enumerate(chunks):
        for g, start in levs:
            for i, d in enumerate(g):
                chunk_of[d] = n_in_chunks + ci
                row_of[d] = start + i

    # per-node bias/response laid out per chunk
    bias_c = np.zeros((128, n_node_chunks), dtype=np.float32)
    resp_c = np.ones((128, n_node_chunks), dtype=np.float32)
    for r in range(NN):
        d = out_idx[r]
        if not needed[d]:
            continue
        bias_c[row_of[d], chunk_of[d] - n_in_chunks] = bias[r]
        resp_c[row_of[d], chunk_of[d] - n_in_chunks] = response[r]

    # weight blocks
    wa_blocks = {}   # (dst_chunk_rel, src_chunk_abs) -> [128,128]
    wb_blocks = {}   # (dst_chunk_rel, group_idx_in_chunk) -> [128,128]
    for r in range(NN):
        d = out_idx[r]
        if not needed[d]:
            continue
        dc = chunk_of[d] - n_in_chunks
        for k in range(K):
            if not valid[r, k]:
                continue
            s = int(in_idx[r, k])
            w = float(edge_w[r, k])
            sc = chunk_of[s]
            if sc == chunk_of[d]:
                gi = next(i for i, (g, st) in enumerate(chunks[dc])
                          if st <= row_of[d] < st + len(g))
                blk = wb_blocks.setdefault((dc, gi), np.zeros((128, 128), np.float32))
            else:
                blk = wa_blocks.setdefault((dc, sc), np.zeros((128, 128), np.float32))
            blk[row_of[s], row_of[d]] += w

    # input digest: when a late chunk reads <=128 distinct input nodes,
    # the host gathers those x columns into one extra v tile so the chunk
    # needs a single input matmul block instead of two
    digest_feeders = None
    dig = n_node_chunks - 1
    if (dig, 0) in wa_blocks and (dig, 1) in wa_blocks:
        b0, b1 = wa_blocks[(dig, 0)], wa_blocks[(dig, 1)]
        f0 = np.where((b0 != 0).any(axis=1))[0]
        f1 = np.where((b1 != 0).any(axis=1))[0]
        if len(f0) + len(f1) <= 128:
            digest_feeders = np.concatenate([f0, 128 + f1]).astype(np.int64)
            wg = np.zeros((128, 128), np.float32)
            wg[0:len(f0)] = b0[f0]
            wg[len(f0):len(f0) + len(f1)] = b1[f1]
            del wa_blocks[(dig, 0)], wa_blocks[(dig, 1)]
            wa_blocks[(dig, -1)] = wg      # src -1 = digest pseudo-chunk

    parta = []   # per node chunk: list of (src_chunk, wa_index), src ascending
    wa_list = []
    for dc in range(n_node_chunks):
        lst = []
        for sc in [-1] + list(range(n_chunks)):
            if (dc, sc) in wa_blocks:
                lst.append((sc, len(wa_list)))
                wa_list.append(wa_blocks[(dc, sc)])
        parta.append(lst)

    partb = []   # per node chunk: list of (group_idx, local_start, m, wb_index|None)
    wb_list = []
    for dc in range(n_node_chunks):
        lst = []
        for gi, (g, st) in enumerate(chunks[dc]):
            if (dc, gi) in wb_blocks:
                lst.append((gi, st, len(g), len(wb_list)))
                wb_list.append(wb_blocks[(dc, gi)])
            else:
                lst.append((gi, st, len(g), None))
        partb.append(lst)

    # output extraction: row-ranges per chunk covering its output nodes
    rows_by_chunk = {}
    for d in output_idx:
        dc = int(chunk_of[d]) - n_in_chunks
        rows_by_chunk.setdefault(dc, []).append(int(row_of[d]))
    out_ranges = []   # (dst_chunk_rel, row0, len, col0)
    col = 0
    pos_of = {}       # (dc, row) -> staged row in o16
    last_oc = max(rows_by_chunk)
    for dc in sorted(rows_by_chunk):
        rows = sorted(set(rows_by_chunk[dc]))
        gap = 10**9 if dc == last_oc else 8
        start = prev = rows[0]
        for r in rows[1:] + [None]:
            if r is not None and r - prev <= gap:
                prev = r
                continue
            ln = prev - start + 1
            out_ranges.append((dc, start, ln, col))
            for rr in range(start, prev + 1):
                pos_of[(dc, rr)] = col + rr - start
            col += ln
            if r is not None:
                start = prev = r
    ntot = col
    assert ntot <= 192, f"staged output rows {ntot} too large"
    colmap = np.zeros(NUM_OUT, dtype=np.int64)
    for oc, d in enumerate(output_idx):
        dc = int(chunk_of[d]) - n_in_chunks
        colmap[oc] = pos_of[(dc, int(row_of[d]))]

    # host-packed weight images: [128, n*128] partition-major so one DMA row
    # per partition is contiguous in DRAM
    def pack(blocks):
        if not blocks:
            return np.zeros((128, 128), np.float16)
        arr = np.stack(blocks).astype(np.float16)        # [n, 128, 128]
        return np.ascontiguousarray(arr.transpose(1, 0, 2).reshape(128, -1))

    return dict(
        out_ranges=out_ranges,
        ntot=ntot,
        colmap=colmap,
        digest_feeders=digest_feeders,
        n_in_chunks=n_in_chunks,
        n_node_chunks=n_node_chunks,
        n_chunks=n_chunks,
        parta=parta,
        partb=partb,
        n_wa=len(wa_list),
        n_wb=len(wb_list),
        wa_pk=pack(wa_list),
        wb_pk=pack(wb_list),
        bias_c=bias_c,
        resp_c=resp_c,
    )


def _build_nc(plan):
    n_in_chunks = plan["n_in_chunks"]
    n_node_chunks = plan["n_node_chunks"]
    n_chunks = plan["n_chunks"]
    parta = plan["parta"]
    partb = plan["partb"]
    out_ranges = plan["out_ranges"]
    ntot = plan["ntot"]
    n_wa = max(plan["n_wa"], 1)
    n_wb = max(plan["n_wb"], 1)

    nc = bacc.Bacc()
    x = nc.dram_tensor("x", [128, 2 * BC], F16, kind="ExternalInput")
    wa = nc.dram_tensor("wa", [128, n_wa * 128], F16, kind="ExternalInput")
    wb = nc.dram_tensor("wb", [128, n_wb * 128], F16, kind="ExternalInput")
    br_d = nc.dram_tensor("br_c", [128, 2 * n_node_chunks], F32, kind="ExternalInput")
    o = nc.dram_tensor("o", [ntot, BC], F16, kind="ExternalOutput")
    feeders = plan["digest_feeders"]
    xg = (nc.dram_tensor("xg", [128, BC], F16, kind="ExternalInput")
          if feeders is not None else None)

    with TileContext(nc) as tc:
        with tc.tile_pool(name="const", bufs=1) as const, \
             tc.tile_pool(name="vpool", bufs=n_chunks) as vpool, \
             tc.tile_pool(name="pc", bufs=4, space="PSUM") as pcp:

            br_sb = const.tile([128, 2 * n_node_chunks], F32, tag="br")
            bias_sb = br_sb[:, 0:n_node_chunks]
            resp_sb = br_sb[:, n_node_chunks:2 * n_node_chunks]
            wa_sb = const.tile([128, n_wa * 128], F16, tag="wa_sb")
            wb_sb = const.tile([128, n_wb * 128], F16, tag="wb_sb")
            zt = const.tile([1, 2], F32, tag="zt")
            wz = const.tile([128, 512], F16, tag="wz")

            # node-major value store: one [128, 4096] tile holds both input
            # chunks (in0|in1 x passA|passB column blocks, host-packed);
            # node chunks are [128, 2048] tiles, plus an optional digest
            vin = vpool.tile([128, 2 * BC], F16, tag="v", name="vin")
            v = {}
            for c in range(n_in_chunks, n_chunks):
                v[c] = vpool.tile([128, BC], F16, tag="v", name=f"v{c}")
            if feeders is not None:
                v[n_chunks] = vpool.tile([128, BC], F16, tag="v", name="vdig")

            def mov(sc, P, q):
                if sc < n_in_chunks:
                    c0 = P * 2 * HALF + sc * HALF + q * 512
                    return vin[:, c0:c0 + 512]
                return v[sc][:, P * HALF + q * 512:P * HALF + (q + 1) * 512]

            # warm-up constants first (gates the PE ramp; memset on the Pool
            # engine so the idle DVE preamble doesn't delay it), then trigger
            # the tanh table load on the scalar engine; bias passed as an AP
            # so no const-pool DMA sits on the dummy act's critical path
            nc.gpsimd.memset(wz[:], 0.0)
            nc.vector.memset(zt[:], 0.0)
            nc.scalar.activation(zt[0:1, 1:2], zt[0:1, 0:1],
                                 mybir.ActivationFunctionType.Tanh,
                                 bias=zt[0:1, 0:1])

            # used-row extent per chunk: matmul contraction is sliced to it,
            # so rows the acts never write are never read either
            csize = [128] * n_in_chunks
            for dc in range(n_node_chunks):
                gi, st, m, bi = partb[dc][-1]
                csize.append(min(((st + m + 31) // 32) * 32, 128))
            if feeders is not None:
                csize.append(min(((len(feeders) + 31) // 32) * 32, 128))

            # ---- DMA schedule, urgency-ordered: chunk-0 weights first, then
            # pass-A input (one DMA per input chunk), bias, chunk-1 weights,
            # pass-B input, remaining weights.
            def dma_x(half):
                nc.sync.dma_start(
                    vin[:, half * BC:(half + 1) * BC],
                    x[:, half * BC:(half + 1) * BC])

            def dma_x_q(half, q):
                b = half * BC + q * 512
                nc.sync.dma_start(
                    vin.rearrange("p (c f) -> p c f", f=512)[:, b // 512::2, :][:, 0:2, :],
                    x.rearrange("p (c f) -> p c f", f=512)[:, b // 512::2, :][:, 0:2, :])

            def wa_span(dc):
                idxs = [ai for _, ai in parta[dc]]
                return (idxs[0], idxs[-1] + 1) if idxs else None

            def wb_span(dc):
                bidx = [bi for _, _, _, bi in partb[dc] if bi is not None]
                return (bidx[0], bidx[-1] + 1) if bidx else None

            def dma_wa(i0, i1):
                nc.sync.dma_start(wa_sb[:, i0 * 128:i1 * 128],
                                  wa[:, i0 * 128:i1 * 128])

            def dma_wb(i0, i1):
                nc.sync.dma_start(wb_sb[:, i0 * 128:i1 * 128],
                                  wb[:, i0 * 128:i1 * 128])

            # pool/SWDGE queue (bypasses the shared HWDGE device): bias and
            # chunk-0 intra weights
            nc.gpsimd.dma_start(br_sb[:], br_d[:])
            s = wb_span(0)
            if s:
                nc.gpsimd.dma_start(wb_sb[:, s[0] * 128:s[1] * 128],
                                    wb[:, s[0] * 128:s[1] * 128])
            # sync/HWDGE queue, urgency-ordered
            dma_x(0)
            s = wa_span(0)
            if s:
                dma_wa(*s)
            dma_x(1)
            s0 = wa_span(1)
            if s0:
                dma_wa(*s0)
            s1 = wb_span(1)
            if s1:
                dma_wb(*s1)
            # remaining weights in pieces
            a_lo = wa_span(2)[0] if n_node_chunks > 2 and wa_span(2) else plan["n_wa"]
            if a_lo < plan["n_wa"]:
                mid = (a_lo + plan["n_wa"] + 1) // 2
                dma_wa(a_lo, mid)
                dma_wa(mid, plan["n_wa"])
            b_lo = wb_span(2)[0] if n_node_chunks > 2 and wb_span(2) else plan["n_wb"]
            if b_lo < plan["n_wb"]:
                dma_wb(b_lo, plan["n_wb"])
            if feeders is not None:
                for h in range(2):
                    nc.sync.dma_start(
                        v[n_chunks][:, h * HALF:(h + 1) * HALF],
                        xg[:, h * HALF:(h + 1) * HALF])

            # ---- PE warm-up: zero matmuls ramp the tensor-engine p-state
            # during the input DMA so the first real matmuls run full speed.
            # psum buffers are managed manually: (stream, chunk) -> buffer
            # (2*chunk+stream) % 4, so a stream's chunk c+2 can start
            # accumulating as soon as its chunk c is fully activated.
            pc_bufs = [pcp.tile([128, HALF], F32, tag="pc", name=f"pcb{i}")
                       for i in range(4)]

            def get_pc(P, dc):
                return pc_bufs[(2 * dc + P) % 4]

            for _ in range(7):
                nc.tensor.matmul(pc_bufs[0][:, 0:512], wz[:, 0:128], wz[:, :],
                                 start=True, stop=False, skip_group_check=True)

            # ---- cascade: the two batch passes (streams A=0, B=1) are
            # zipped at LEVEL granularity over the same chunk, B lagging one
            # level: A_L0, A_L1, B_L0, A_L2, B_L1, ..., B_L(last).  The act
            # queue is in-order, so alternating independent streams hides
            # each act's sem+matmul dependency path (~780ns) under the other
            # stream's 1038ns activation, keeping the scalar engine at full
            # cadence even when dependency-bound.
            started = {}                   # (P, dc, q) -> bool

            # filler units: one 512-wide cross-chunk matmul each.
            # ready_c = chunk pair after which the source values exist.
            units = []                     # [ready_c, dst_c, P, sc, ai, q]
            for dc in range(n_node_chunks):
                for P in range(2):
                    for sc, ai in parta[dc]:
                        ready = 0 if sc < n_in_chunks else sc - n_in_chunks + 1
                        for q in range(NQ):
                            units.append((ready, dc, P, sc, ai, q))
            units = [(r, d, P, (n_chunks if sc == -1 else sc), ai, q)
                     for r, d, P, sc, ai, q in units]
            units.sort(key=lambda u: (u[1], u[2], u[5], u[3]))
            emitted = [False] * len(units)
            acts_done = [[False] * n_node_chunks for _ in range(2)]

            def emit_unit(i):
                ready, dcu, P, sc, ai, q = units[i]
                pcu = get_pc(P, dcu)
                ks = csize[sc]
                nc.tensor.matmul(
                    pcu[:, q * 512:(q + 1) * 512],
                    wa_sb[0:ks, ai * 128:(ai + 1) * 128],
                    mov(sc, P, q)[0:ks, :],
                    start=not started.get((P, dcu, q), False), stop=False,
                    skip_group_check=True)
                started[(P, dcu, q)] = True
                emitted[i] = True

            def src_ready(i):
                r = units[i][0]
                return r == 0 or acts_done[units[i][2]][r - 1]

            def act_slice(dc_, gi_):
                gi, st, m, bi = partb[dc_][gi_]
                al = (st // 32) * 32
                if st + m <= al + 32:
                    return al, min(al + 32, 128)
                return 0, min(((st + m + 31) // 32) * 32, 128)

            def emit_fillers(cur_c, budget):
                n = 0
                for i in range(len(units)):
                    if n >= budget:
                        break
                    if emitted[i] or not src_ready(i):
                        continue
                    dst, P = units[i][1], units[i][2]
                    # psum buffer for (P, dst) conflicts with (P, dst-2):
                    # that chunk's acts must all be emitted first
                    if dst > cur_c + 1 and not (dst == cur_c + 2
                                                and acts_done[P][dst - 2]):
                        continue
                    emit_unit(i)
                    n += 1

            def emit_act(P, dc, gi_idx):
                gc = n_in_chunks + dc
                gi, st, m, bi = partb[dc][gi_idx]
                pcu = get_pc(P, dc)
                # fillers first: they sit BEFORE this slot's deadline-critical
                # wb/forced matmuls in the in-order PE queue
                emit_fillers(dc, 3)
                if gi_idx == 0:
                    # all cross-chunk contributions must be in
                    for i in range(len(units)):
                        if not emitted[i] and units[i][1] == dc and units[i][2] == P:
                            emit_unit(i)
                if bi is not None:
                    # only rows written by earlier levels of this chunk are
                    # readable (act slices cover [0, prefix-max) contiguously)
                    ks = max(act_slice(dc, j)[1] for j in range(gi_idx))
                    wt = wb_sb[0:ks, bi * 128:(bi + 1) * 128]
                    for q in range(NQ):
                        nc.tensor.matmul(
                            pcu[:, q * 512:(q + 1) * 512],
                            wt,
                            v[gc][0:ks, P * HALF + q * 512:P * HALF + (q + 1) * 512],
                            start=False, stop=False,
                            skip_group_check=True)
                # per-level tanh: PSUM partition access must start 32-aligned
                # and stay within one 32-quadrant (unless starting at row 0),
                # so activate the level's full quadrant, or [0, end) rounded
                # up when it spans quadrants.  Extra rows recompute earlier
                # values or write garbage that later levels overwrite.
                lo_r, hi_r = act_slice(dc, gi_idx)
                nc.scalar.activation(
                    v[gc][lo_r:hi_r, P * HALF:(P + 1) * HALF],
                    pcu[lo_r:hi_r, :],
                    mybir.ActivationFunctionType.Tanh,
                    bias=bias_sb[lo_r:hi_r, dc:dc + 1],
                    scale=resp_sb[lo_r:hi_r, dc:dc + 1])
                if gi_idx == len(partb[dc]) - 1:
                    acts_done[P][dc] = True
                # last chunk: stream final output rows once no later act
                # slice can rewrite them, keeping the post-last-act DMA tiny
                if dc == n_node_chunks - 1 and P == 1:
                    nlev = len(partb[dc])
                    flush_hi = 128 if gi_idx == nlev - 1 else min(
                        act_slice(dc, j)[0] for j in range(gi_idx + 1, nlev))
                    rgs = [(r0, ln, c0) for c, r0, ln, c0 in out_ranges
                           if c == dc]
                    for r0, ln, c0 in rgs:
                        lo = max(r0, oflush[0])
                        hi = min(r0 + ln, flush_hi)
                        if lo < hi:
                            nc.sync.dma_start(
                                o[c0 + lo - r0:c0 + hi - r0, :],
                                v[gc][lo:hi, :])
                    oflush[0] = max(oflush[0], flush_hi)

            oq = [0]
            oflush = [0]
            # global slot list: stream B staggered one act behind stream A,
            # so adjacent acts always come from independent streams and each
            # act's sem+matmul dependency path hides under the other stream
            slots = []
            for dc in range(n_node_chunks):
                for i in range(len(partb[dc])):
                    slots.append((0, dc, i))
                    slots.append((1, dc, i))
            for P, dc, gi_idx in slots:
                emit_act(P, dc, gi_idx)
                # stream a finished chunk's output rows out
                if (P == 1 and gi_idx == len(partb[dc]) - 1
                        and dc < n_node_chunks - 1):
                    rgs = [(r0, ln, c0) for c, r0, ln, c0 in out_ranges
                           if c == dc]
                    for r0, ln, c0 in rgs:
                        eng = nc.sync if oq[0] % 2 == 0 else nc.scalar
                        oq[0] += 1
                        eng.dma_start(o[c0:c0 + ln, :],
                                      v[n_in_chunks + dc][r0:r0 + ln, :])

    nc.compile()
    return nc


_CACHE = {}


def _get_compiled(key, plan):
    if key not in _CACHE:
        _CACHE[key] = _build_nc(plan)
    return _CACHE[key]


def kernel(inputs, edge_w, bias, response, in_idx, edge_mask, out_idx, output_idx):
    inputs = np.ascontiguousarray(np.asarray(inputs, dtype=np.float32))
    plan = _plan(in_idx, edge_mask, edge_w, bias, response, out_idx, output_idx)

    key = (plan["wa_pk"].tobytes(), plan["wb_pk"].tobytes(),
           str(plan["out_ranges"]), plan["bias_c"].tobytes(),
           plan["resp_c"].tobytes())
    nc = _get_compiled(hash(key), plan)

    base = {
        "wa": plan["wa_pk"],
        "wb": plan["wb_pk"],
        "br_c": np.ascontiguousarray(
            np.concatenate([plan["bias_c"], plan["resp_c"]], axis=1)),
    }

    x16 = inputs.astype(np.float16)
    feeders = plan["digest_feeders"]
    in_maps = []
    for c in range(NCORES):
        m = dict(base)
        # node-major transpose packed as [128, in0A|in1A|in0B|in1B]
        xt = x16[c * BC:(c + 1) * BC].T
        xh = np.empty((128, 2 * BC), np.float16)
        xh[:, 0:HALF] = xt[0:128, 0:HALF]
        xh[:, HALF:2 * HALF] = xt[128:256, 0:HALF]
        xh[:, 2 * HALF:3 * HALF] = xt[0:128, HALF:2 * HALF]
        xh[:, 3 * HALF:4 * HALF] = xt[128:256, HALF:2 * HALF]
        m["x"] = xh
        if feeders is not None:
            g = np.zeros((128, BC), np.float16)
            g[0:len(feeders)] = xt[feeders]
            m["xg"] = g
        in_maps.append(m)

    res = run_bass_kernel_spmd(nc, in_maps, core_ids=list(range(NCORES)))
    kernel.last_results = res
    colmap = np.asarray(plan["colmap"])
    out = np.concatenate(
        [res.results[c]["o"][colmap].T for c in range(NCORES)], axis=0)
    return np.ascontiguousarray(out.astype(np.float32))


kernel.last_results = None
